# revision 2
# baseline (speedup 1.0000x reference)
"""KPCNN (kernel-predicting CNN) Trainium2 Bass kernel.

Strategy (hardcoded for B=32768, 8 cores, pure data parallel, 4096 samples/core):
 - All convs on 5x5 spatial are reformulated as dense matmuls over flattened
   (pixel, channel) feature vectors, row-banded by output image row so each
   125-wide output chunk contracts only the 2-3 input row chunks in its 3x3
   band (13 [125x125] blocks per 25->25 layer instead of 25).
 - Activations are feature-major [feat, batch] in SBUF, fp32r (TF32-like,
   full PE rate at N>=256), PSUM accumulate fp32.
 - Entry: PE-transpose of naturally-DMA'd [128 samples, 200 feat] tiles.
 - Tail (softmax over 6 predicted weights + per-pixel color mix) runs
   sample-major after PE-transposing back, on DVE/ACT.
Weight densification happens on host (weights are tiny).

Host dispatch: a cached jax.jit(shard_map(bass_exec)) callable built once per
process (the library's run_bass_kernel_spmd rebuilds the closure per call,
forcing a full retrace + XLA + BIR->NEFF recompile every invocation). Weights
are densified once and kept device-resident; zero output buffers are created
on-device. A warm call only ships the input batch down and the result back.
"""
import sys
sys.path.insert(0, '/opt/trn_rl_repo')
import os
import time
import hashlib
import numpy as np

B_TOTAL = 32768
N_CORES = 8
N_PER_CORE = B_TOTAL // N_CORES   # 4096
NT = 512                          # samples per supertile
N_ST = N_PER_CORE // NT           # 8 supertiles
NUM_MID = 6

_CACHE = {}
_TIMING = os.environ.get("KERNEL_TIMING", "") != ""


def _tlog(msg, t0):
    if _TIMING:
        print(f"[kernel.py {time.time()-t0:8.3f}s] {msg}", file=sys.stderr, flush=True)


def _band(y):
    return [yi for yi in (y - 1, y, y + 1) if 0 <= yi <= 4]


def _densify_mid(w):
    """w [25,25,3,3] OIHW -> [13,125,125] blocks (lhsT: [q_in, q_out])."""
    blocks = np.zeros((13, 125, 125), np.float32)
    bi = 0
    for y_out in range(5):
        for y_in in _band(y_out):
            dy = y_in - y_out
            for x_in in range(5):
                for x_out in range(5):
                    dx = x_in - x_out
                    if abs(dx) <= 1:
                        blocks[bi, x_in*25:(x_in+1)*25, x_out*25:(x_out+1)*25] = \
                            w[:, :, dy+1, dx+1].T
            bi += 1
    return blocks


def _densify_conv0(w):
    """w [25,8,3,3] -> [200,625]: row c_in*25+y_in*5+x_in, col y_out*125+x_out*25+c_out."""
    W = np.zeros((200, 625), np.float32)
    for y_in in range(5):
        for x_in in range(5):
            for y_out in range(5):
                dy = y_in - y_out
                if abs(dy) > 1:
                    continue
                for x_out in range(5):
                    dx = x_in - x_out
                    if abs(dx) > 1:
                        continue
                    for c_in in range(8):
                        W[c_in*25 + y_in*5 + x_in,
                          y_out*125 + x_out*25:y_out*125 + x_out*25 + 25] = \
                            w[:, c_in, dy+1, dx+1]
    return W


def _densify_last(w):
    """w [6,25,3,3] -> [625,150]: row y_in*125+x_in*25+c_in, col wi*25+y_out*5+x_out."""
    W = np.zeros((625, 150), np.float32)
    for y_in in range(5):
        for x_in in range(5):
            for y_out in range(5):
                dy = y_in - y_out
                if abs(dy) > 1:
                    continue
                for x_out in range(5):
                    dx = x_in - x_out
                    if abs(dx) > 1:
                        continue
                    for c_in in range(25):
                        for wi in range(6):
                            W[y_in*125 + x_in*25 + c_in, wi*25 + y_out*5 + x_out] = \
                                w[wi, c_in, dy+1, dx+1]
    return W


def _build():
    import concourse.bass as bass
    from concourse import bacc
    import concourse.tile as tile
    import concourse.mybir as mybir

    dt = mybir.dt
    AF = mybir.ActivationFunctionType
    ALU = mybir.AluOpType

    nc = bacc.Bacc("TRN2", target_bir_lowering=False, debug=False)

    f32, f32r = dt.float32, dt.float32r
    x_d = nc.dram_tensor("x", [N_PER_CORE, 200], f32, kind="ExternalInput").ap()
    y_d = nc.dram_tensor("y", [N_PER_CORE, 75], f32, kind="ExternalOutput").ap()
    w0a_d = nc.dram_tensor("w0a", [128, 625], f32, kind="ExternalInput").ap()
    w0b_d = nc.dram_tensor("w0b", [72, 625], f32, kind="ExternalInput").ap()
    wm_d = nc.dram_tensor("wm", [125, NUM_MID, 13, 125], f32, kind="ExternalInput").ap()
    wl_d = nc.dram_tensor("wl", [125, 5, 150], f32, kind="ExternalInput").ap()
    wp_d = nc.dram_tensor("wp", [75, 18], f32, kind="ExternalInput").ap()
    id_d = nc.dram_tensor("ident", [128, 128], f32, kind="ExternalInput").ap()
    b0_d = nc.dram_tensor("b0q", [125, 1], f32, kind="ExternalInput").ap()
    bm_d = nc.dram_tensor("bmq", [125, NUM_MID], f32, kind="ExternalInput").ap()
    bl_d = nc.dram_tensor("blq", [75, 2], f32, kind="ExternalInput").ap()
    bp_d = nc.dram_tensor("bpq", [18, 1], f32, kind="ExternalInput").ap()

    with tile.TileContext(nc) as tc:
        with tc.tile_pool(name="wpool", bufs=1) as wpool, \
             tc.tile_pool(name="apool", bufs=3) as apool, \
             tc.tile_pool(name="npool", bufs=6) as npool, \
             tc.tile_pool(name="tpool", bufs=6) as tpool, \
             tc.tile_pool(name="pspool", bufs=8, space="PSUM") as pspool:

            w0a = wpool.tile([128, 625], f32r)
            w0b = wpool.tile([72, 625], f32r)
            wm = wpool.tile([125, NUM_MID, 13, 125], f32r)
            wl = wpool.tile([125, 5, 150], f32r)
            wp = wpool.tile([75, 18], f32r)
            ident = wpool.tile([128, 128], f32r)
            b0q = wpool.tile([125, 1], f32)
            bmq = wpool.tile([125, NUM_MID], f32)
            blq = wpool.tile([75, 2], f32)
            bpq = wpool.tile([18, 1], f32)
            nc.sync.dma_start(out=w0a, in_=w0a_d.bitcast(f32r))
            nc.sync.dma_start(out=w0b, in_=w0b_d.bitcast(f32r))
            nc.sync.dma_start(out=wm, in_=wm_d.bitcast(f32r))
            nc.sync.dma_start(out=wl, in_=wl_d.bitcast(f32r))
            nc.sync.dma_start(out=wp, in_=wp_d.bitcast(f32r))
            nc.sync.dma_start(out=ident, in_=id_d.bitcast(f32r))
            nc.sync.dma_start(out=b0q, in_=b0_d)
            nc.sync.dma_start(out=bmq, in_=bm_d)
            nc.sync.dma_start(out=blq, in_=bl_d)
            nc.sync.dma_start(out=bpq, in_=bp_d)

            for s in range(N_ST):
                base = s * NT
                # --- entry: DMA natural tiles, PE-transpose to feature-major
                xA = apool.tile([128, NT], f32r)
                xB = apool.tile([72, NT], f32r)
                for g in range(4):
                    nat = npool.tile([128, 200], f32r, tag="nat")
                    nc.sync.dma_start(
                        out=nat, in_=x_d[base+g*128:base+(g+1)*128, :].bitcast(f32r))
                    psA = pspool.tile([128, 128], f32r, tag="ps")
                    nc.tensor.transpose(psA, nat[:, 0:128], ident)
                    nc.vector.tensor_copy(xA[:, g*128:(g+1)*128], psA)
                    psB = pspool.tile([72, 128], f32r, tag="ps")
                    nc.tensor.transpose(psB, nat[:, 128:200], ident)
                    nc.vector.tensor_copy(xB[:, g*128:(g+1)*128], psB)

                # --- conv0 (dense 200->625)
                h = apool.tile([125, 5, NT], f32r, tag="h")
                for y in range(5):
                    ps = pspool.tile([125, NT], f32, tag="ps")
                    nc.tensor.matmul(ps, w0a[:, y*125:(y+1)*125], xA,
                                     start=True, stop=False)
                    nc.tensor.matmul(ps, w0b[:, y*125:(y+1)*125], xB,
                                     start=False, stop=True)
                    if y >= 3:  # balance eviction load ACT vs DVE
                        nc.vector.tensor_scalar(h[:, y, :], ps, b0q, 0.0,
                                                op0=ALU.add, op1=ALU.max)
                    else:
                        nc.scalar.activation(h[:, y, :], ps, AF.Relu, bias=b0q)

                # --- 6 mid layers (row-banded 625->625)
                for l in range(NUM_MID):
                    hn = apool.tile([125, 5, NT], f32r, tag="h")
                    for y in range(5):
                        bnd = _band(y)
                        bi = sum(len(_band(yy)) for yy in range(y))
                        ps = pspool.tile([125, NT], f32, tag="ps")
                        for j, y_in in enumerate(bnd):
                            nc.tensor.matmul(ps, wm[:, l, bi+j, :], h[:, y_in, :],
                                             start=(j == 0), stop=(j == len(bnd)-1))
                        if y >= 3:
                            nc.vector.tensor_scalar(hn[:, y, :], ps,
                                                    bmq[:, l:l+1], 0.0,
                                                    op0=ALU.add, op1=ALU.max)
                        else:
                            nc.scalar.activation(hn[:, y, :], ps, AF.Relu,
                                                 bias=bmq[:, l:l+1])
                    h = hn

                # --- last layer (625->150, logits, w-major cols)
                hl = apool.tile([75, 2, NT], f32r)
                for m in range(2):
                    ps = pspool.tile([75, NT], f32, tag="ps")
                    for k in range(5):
                        nc.tensor.matmul(ps, wl[:, k, m*75:(m+1)*75], h[:, k, :],
                                         start=(k == 0), stop=(k == 4))
                    nc.scalar.activation(hl[:, m, :], ps, AF.Identity,
                                         bias=blq[:, m:m+1])

                # --- post conv (colors: 75->18)
                colors = apool.tile([18, NT], f32r)
                psc = pspool.tile([18, NT], f32, tag="ps")
                nc.tensor.matmul(psc, wp, xA[0:75, :], start=True, stop=True)
                nc.scalar.activation(colors, psc, AF.Identity, bias=bpq)

                # --- tail: per 128-group, sample-major softmax + color mix
                for g in range(4):
                    gs = slice(g*128, (g+1)*128)
                    # fp32r matmul ISA restriction: innermost free n_step must
                    # be even on moving operand and dst -> pad 75 to 76.
                    tE0 = pspool.tile([128, 76], f32r, tag="ps")
                    nc.tensor.transpose(tE0, hl[:, 0, gs], ident[0:75, 0:76])
                    tE1 = pspool.tile([128, 76], f32r, tag="ps")
                    nc.tensor.transpose(tE1, hl[:, 1, gs], ident[0:75, 0:76])
                    E = tpool.tile([128, 150], f32, tag="E")
                    nc.scalar.activation(E[:, 0:75], tE0[:, 0:75], AF.Exp)
                    nc.scalar.activation(E[:, 75:150], tE1[:, 0:75], AF.Exp)
                    tC = pspool.tile([128, 18], f32r, tag="ps")
                    nc.tensor.transpose(tC, colors[:, gs], ident[0:18, 0:18])
                    colT = tpool.tile([128, 18], f32, tag="colT")
                    nc.scalar.activation(colT, tC, AF.Copy)

                    S = tpool.tile([128, 25], f32, tag="S")
                    nc.vector.tensor_reduce(
                        out=S, in_=E.rearrange("p (w q) -> p q w", w=6),
                        axis=mybir.AxisListType.X, op=ALU.add)
                    R = tpool.tile([128, 25], f32, tag="R")
                    nc.vector.reciprocal(R, S)

                    U = tpool.tile([128, 3, 25], f32, tag="U")
                    for c in range(3):
                        nc.vector.tensor_scalar_mul(
                            U[:, c, :], E[:, 0:25], colT[:, c*6:c*6+1])
                        for w in range(1, 6):
                            nc.vector.scalar_tensor_tensor(
                                out=U[:, c, :], in0=E[:, w*25:(w+1)*25],
                                scalar=colT[:, c*6+w:c*6+w+1], in1=U[:, c, :],
                                op0=ALU.mult, op1=ALU.add)
                    F = tpool.tile([128, 3, 25], f32, tag="F")
                    nc.vector.tensor_tensor(
                        out=F, in0=U,
                        in1=R.unsqueeze(1).broadcast_to([128, 3, 25]),
                        op=ALU.mult)
                    nc.sync.dma_start(
                        out=y_d[base+g*128:base+(g+1)*128, :],
                        in_=F.rearrange("p a b -> p (a b)"))

    nc.compile()
    return nc


def _prep_weights(w0, b0, wmid, bmid, wlast, blast, wpost, bpost):
    W0 = _densify_conv0(np.asarray(w0, np.float32))
    wm = np.zeros((125, NUM_MID, 13, 125), np.float32)
    for l in range(NUM_MID):
        blocks = _densify_mid(np.asarray(wmid[l], np.float32))
        for bi in range(13):
            wm[:, l, bi, :] = blocks[bi]
    Wl = _densify_last(np.asarray(wlast, np.float32))
    wl = np.ascontiguousarray(
        np.transpose(Wl.reshape(5, 125, 150), (1, 0, 2)))
    wp = np.ascontiguousarray(
        np.asarray(wpost, np.float32).reshape(18, 75).T)
    b0q = np.tile(np.asarray(b0, np.float32), 5)[:, None]
    bmq = np.stack([np.tile(np.asarray(bmid[l], np.float32), 5)
                    for l in range(NUM_MID)], axis=1)
    blq = np.asarray(blast, np.float32).repeat(25).reshape(2, 75).T
    bpq = np.asarray(bpost, np.float32)[:, None]
    return {
        "w0a": np.ascontiguousarray(W0[0:128]),
        "w0b": np.ascontiguousarray(W0[128:200]),
        "wm": wm, "wl": wl, "wp": wp,
        "ident": np.eye(128, dtype=np.float32),
        "b0q": np.ascontiguousarray(b0q), "bmq": np.ascontiguousarray(bmq),
        "blq": np.ascontiguousarray(blq), "bpq": bpq,
    }


def _get_ctx():
    """Build (once per process) the Bass module and a STABLE jitted runner.

    The library's run_bass_kernel_spmd/run_bass_via_pjrt constructs a fresh
    _body closure per call, so jax.jit retraces and recompiles (XLA +
    BIR->NEFF) on every invocation. Here the jit closure lives in _CACHE, so
    warm calls hit jax's C++ fast path and only pay data movement.
    """
    if "ctx" in _CACHE:
        return _CACHE["ctx"]

    import jax
    import jax.numpy as jnp
    from jax.sharding import Mesh, PartitionSpec, NamedSharding
    from jax.experimental.shard_map import shard_map
    from concourse import bass2jax
    import concourse.mybir as mybir

    nc = _build()
    bass2jax.install_neuronx_cc_hook()
    assert nc.dbg_addr is None, "built with debug=False"

    partition_name = (nc.partition_id_tensor.name
                      if nc.partition_id_tensor is not None else None)
    in_names, out_names, out_avals = [], [], []
    for alloc in nc.m.functions[0].allocations:
        if not isinstance(alloc, mybir.MemoryLocationSet):
            continue
        name = alloc.memorylocations[0].name
        if alloc.kind == "ExternalInput":
            if name != partition_name:
                in_names.append(name)
        elif alloc.kind == "ExternalOutput":
            out_names.append(name)
            shape = tuple(alloc.tensor_shape)
            dtype = mybir.dt.np(alloc.dtype)
            out_avals.append(jax.core.ShapedArray(shape, dtype))
    n_params = len(in_names)
    n_outs = len(out_names)
    all_in_names = list(in_names) + list(out_names)
    if partition_name is not None:
        all_in_names.append(partition_name)
    donate = tuple(range(n_params, n_params + n_outs))

    def _body(*args):
        operands = list(args)
        if partition_name is not None:
            operands.append(bass2jax.partition_id_tensor())
        outs = bass2jax._bass_exec_p.bind(
            *operands,
            out_avals=tuple(out_avals),
            in_names=tuple(all_in_names),
            out_names=tuple(out_names),
            lowering_input_output_aliases=(),
            sim_require_finite=True,
            sim_require_nnan=True,
            nc=nc,
        )
        return tuple(outs)

    devices = jax.devices()[:N_CORES]
    assert len(devices) == N_CORES
    mesh = Mesh(np.asarray(devices), ("core",))
    in_specs = (PartitionSpec("core"),) * (n_params + n_outs)
    out_specs = (PartitionSpec("core"),) * n_outs
    runner = jax.jit(
        shard_map(_body, mesh=mesh, in_specs=in_specs,
                  out_specs=out_specs, check_rep=False),
        donate_argnums=donate, keep_unused=True)
    shard = NamedSharding(mesh, PartitionSpec("core"))
    zmakers = [
        jax.jit(
            (lambda aval: lambda: jnp.zeros(
                (N_CORES * aval.shape[0],) + tuple(aval.shape[1:]), aval.dtype
            ))(a),
            out_shardings=shard)
        for a in out_avals
    ]
    ctx = {
        "nc": nc, "runner": runner, "zmakers": zmakers, "shard": shard,
        "in_names": in_names, "out_names": out_names, "out_avals": out_avals,
        "jax": jax,
    }
    _CACHE["ctx"] = ctx
    return ctx


def _stage_weights(ctx, wmap):
    """Replicate each weight across the 8 cores as a device-resident global
    array (sharded concat on axis 0), uploaded once and reused every call."""
    jax = ctx["jax"]
    staged = {}
    for k, v in wmap.items():
        g = np.ascontiguousarray(
            np.broadcast_to(v[None], (N_CORES,) + v.shape).reshape(
                (N_CORES * v.shape[0],) + v.shape[1:]))
        staged[k] = jax.device_put(g, ctx["shard"])
    for a in staged.values():
        a.block_until_ready()
    return staged


def _weight_key(*arrs):
    h = hashlib.blake2b(digest_size=16)
    for a in arrs:
        h.update(np.ascontiguousarray(np.asarray(a, np.float32)).tobytes())
    return h.digest()


def kernel(input, w0, b0, wmid, bmid, wlast, blast, wpost, bpost, _trace=False):
    t0 = time.time()
    if _trace:
        return _kernel_traced(input, w0, b0, wmid, bmid, wlast, blast,
                              wpost, bpost)
    ctx = _get_ctx()
    _tlog("ctx ready", t0)

    key = _weight_key(w0, b0, wmid, bmid, wlast, blast, wpost, bpost)
    if _CACHE.get("wkey") != key:
        wmap = _prep_weights(w0, b0, wmid, bmid, wlast, blast, wpost, bpost)
        _tlog("weights densified", t0)
        _CACHE["weights"] = _stage_weights(ctx, wmap)
        _CACHE["wkey"] = key
        _tlog("weights staged to devices", t0)
    staged = _CACHE["weights"]

    x = np.ascontiguousarray(
        np.asarray(input, np.float32).reshape(B_TOTAL, 200))
    _tlog("input marshaled", t0)

    zeros = [zm() for zm in ctx["zmakers"]]
    args = [x if name == "x" else staged[name] for name in ctx["in_names"]]
    out_arrs = ctx["runner"](*args, *zeros)
    _tlog("runner dispatched", t0)
    out = np.asarray(out_arrs[0])
    _tlog("output fetched", t0)
    return out.reshape(B_TOTAL, 3, 5, 5)


def _kernel_traced(input, w0, b0, wmid, bmid, wlast, blast, wpost, bpost):
    """Legacy library path (per-call compile) — only used for --trace runs."""
    from concourse import bass_utils
    if "nc_trace" not in _CACHE:
        _CACHE["nc_trace"] = _build()
    nc = _CACHE["nc_trace"]
    wmap = _prep_weights(w0, b0, wmid, bmid, wlast, blast, wpost, bpost)
    x = np.ascontiguousarray(np.asarray(input, np.float32).reshape(B_TOTAL, 200))
    in_maps = []
    for c in range(N_CORES):
        m = dict(wmap)
        m["x"] = np.ascontiguousarray(x[c*N_PER_CORE:(c+1)*N_PER_CORE])
        in_maps.append(m)
    res = bass_utils.run_bass_kernel_spmd(
        nc, in_maps, core_ids=list(range(N_CORES)), trace=True)
    out = np.concatenate([res.results[c]["y"] for c in range(N_CORES)], axis=0)
    _CACHE["last_result"] = res
    return out.reshape(B_TOTAL, 3, 5, 5)


# revision 6
# speedup vs baseline: 3.9260x; 3.9260x over previous
"""KPCNN (kernel-predicting CNN) Trainium2 Bass kernel.

Device strategy (B=32768, 8 cores, pure data parallel):
 - All convs on 5x5 spatial are reformulated as dense matmuls over flattened
   (pixel, channel) feature vectors, row-banded by output image row so each
   125-wide output chunk contracts only the 2-3 input row chunks in its 3x3
   band (13 [125x125] blocks per 25->25 layer instead of 25).
 - Activations are feature-major [feat, batch] in SBUF, fp32r (TF32-like,
   full PE rate at N>=256), PSUM accumulate fp32.
 - Entry: PE-transpose of naturally-DMA'd [128 samples, 200 feat] fp16 tiles.
 - Tail (softmax over 6 predicted weights + per-pixel color mix) runs
   sample-major after PE-transposing back, on DVE/ACT; result stored fp16.

Host dispatch (the actual bottleneck — the axon tunnel moves ~49MB/s H2D,
~30MB/s D2H, with ~70ms RTT per synchronous dispatch):
 - One STABLE jax.jit(shard_map(bass_exec)) built per process and cached
   (the library's run_bass_kernel_spmd rebuilds its closure per call, which
   forces a retrace + XLA + BIR->NEFF recompile on every invocation).
 - Weights are densified once, replicated, and kept device-resident.
 - I/O travels as fp16 (input 13MB down, output 4.9MB up instead of 26/9.8
   fp32); fp16's 10-bit mantissa matches the fp32r compute precision.
 - The NEFF's output buffer is fully written by the kernel, so the zero
   output-donation buffers the library path re-ships per call are created
   on-device once and reused (no donation).
 - Optional batch chunking (KERNEL_CHUNKS) pipelines H2D/exec/D2H.
"""
import sys
sys.path.insert(0, '/opt/trn_rl_repo')
import os
import time
import hashlib
import numpy as np

B_TOTAL = 32768
N_CORES = 8
N_PER_CORE = B_TOTAL // N_CORES   # 4096
NT = 512                          # samples per supertile
NUM_MID = 6
CHUNKS = int(os.environ.get("KERNEL_CHUNKS", "1"))
NPC = N_PER_CORE // CHUNKS        # samples per core per chunk
B_CHUNK = B_TOTAL // CHUNKS

_CACHE = {}
_TIMING = os.environ.get("KERNEL_TIMING", "") != ""


def _tlog(msg, t0):
    if _TIMING:
        print(f"[kernel.py {time.time()-t0:8.3f}s] {msg}", file=sys.stderr, flush=True)


def _band(y):
    return [yi for yi in (y - 1, y, y + 1) if 0 <= yi <= 4]


def _densify_mid(w):
    """w [25,25,3,3] OIHW -> [13,125,125] blocks (lhsT: [q_in, q_out])."""
    blocks = np.zeros((13, 125, 125), np.float32)
    bi = 0
    for y_out in range(5):
        for y_in in _band(y_out):
            dy = y_in - y_out
            for x_in in range(5):
                for x_out in range(5):
                    dx = x_in - x_out
                    if abs(dx) <= 1:
                        blocks[bi, x_in*25:(x_in+1)*25, x_out*25:(x_out+1)*25] = \
                            w[:, :, dy+1, dx+1].T
            bi += 1
    return blocks


def _densify_conv0(w):
    """w [25,8,3,3] -> [200,625]: row c_in*25+y_in*5+x_in, col y_out*125+x_out*25+c_out."""
    W = np.zeros((200, 625), np.float32)
    for y_in in range(5):
        for x_in in range(5):
            for y_out in range(5):
                dy = y_in - y_out
                if abs(dy) > 1:
                    continue
                for x_out in range(5):
                    dx = x_in - x_out
                    if abs(dx) > 1:
                        continue
                    for c_in in range(8):
                        W[c_in*25 + y_in*5 + x_in,
                          y_out*125 + x_out*25:y_out*125 + x_out*25 + 25] = \
                            w[:, c_in, dy+1, dx+1]
    return W


def _densify_last(w):
    """w [6,25,3,3] -> [625,150]: row y_in*125+x_in*25+c_in, col wi*25+y_out*5+x_out."""
    W = np.zeros((625, 150), np.float32)
    for y_in in range(5):
        for x_in in range(5):
            for y_out in range(5):
                dy = y_in - y_out
                if abs(dy) > 1:
                    continue
                for x_out in range(5):
                    dx = x_in - x_out
                    if abs(dx) > 1:
                        continue
                    for c_in in range(25):
                        for wi in range(6):
                            W[y_in*125 + x_in*25 + c_in, wi*25 + y_out*5 + x_out] = \
                                w[wi, c_in, dy+1, dx+1]
    return W


def _build(npc):
    import concourse.bass as bass
    from concourse import bacc
    import concourse.tile as tile
    import concourse.mybir as mybir

    dt = mybir.dt
    AF = mybir.ActivationFunctionType
    ALU = mybir.AluOpType

    nc = bacc.Bacc("TRN2", target_bir_lowering=False, debug=False)

    f32, f32r, f16 = dt.float32, dt.float32r, dt.float16
    n_st = npc // NT
    assert npc % NT == 0
    x_d = nc.dram_tensor("x", [npc, 200], f16, kind="ExternalInput").ap()
    y_d = nc.dram_tensor("y", [npc, 75], f16, kind="ExternalOutput").ap()
    w0a_d = nc.dram_tensor("w0a", [128, 625], f32, kind="ExternalInput").ap()
    w0b_d = nc.dram_tensor("w0b", [72, 625], f32, kind="ExternalInput").ap()
    wm_d = nc.dram_tensor("wm", [125, NUM_MID, 13, 125], f32, kind="ExternalInput").ap()
    wl_d = nc.dram_tensor("wl", [125, 5, 150], f32, kind="ExternalInput").ap()
    wp_d = nc.dram_tensor("wp", [75, 18], f32, kind="ExternalInput").ap()
    id_d = nc.dram_tensor("ident", [128, 128], f32, kind="ExternalInput").ap()
    idh_d = nc.dram_tensor("identh", [128, 128], f16, kind="ExternalInput").ap()
    b0_d = nc.dram_tensor("b0q", [125, 1], f32, kind="ExternalInput").ap()
    bm_d = nc.dram_tensor("bmq", [125, NUM_MID], f32, kind="ExternalInput").ap()
    bl_d = nc.dram_tensor("blq", [75, 2], f32, kind="ExternalInput").ap()
    bp_d = nc.dram_tensor("bpq", [18, 1], f32, kind="ExternalInput").ap()

    with tile.TileContext(nc) as tc:
        with tc.tile_pool(name="wpool", bufs=1) as wpool, \
             tc.tile_pool(name="apool", bufs=3) as apool, \
             tc.tile_pool(name="npool", bufs=6) as npool, \
             tc.tile_pool(name="tpool", bufs=6) as tpool, \
             tc.tile_pool(name="pspool", bufs=8, space="PSUM") as pspool:

        # --- load weights (resident in SBUF for the whole kernel)
            w0a = wpool.tile([128, 625], f32r)
            w0b = wpool.tile([72, 625], f32r)
            wm = wpool.tile([125, NUM_MID, 13, 125], f32r)
            wl = wpool.tile([125, 5, 150], f32r)
            wp = wpool.tile([75, 18], f32r)
            ident = wpool.tile([128, 128], f32r)
            identh = wpool.tile([128, 128], f16)
            b0q = wpool.tile([125, 1], f32)
            bmq = wpool.tile([125, NUM_MID], f32)
            blq = wpool.tile([75, 2], f32)
            bpq = wpool.tile([18, 1], f32)
            nc.sync.dma_start(out=w0a, in_=w0a_d.bitcast(f32r))
            nc.sync.dma_start(out=w0b, in_=w0b_d.bitcast(f32r))
            nc.sync.dma_start(out=wm, in_=wm_d.bitcast(f32r))
            nc.sync.dma_start(out=wl, in_=wl_d.bitcast(f32r))
            nc.sync.dma_start(out=wp, in_=wp_d.bitcast(f32r))
            nc.sync.dma_start(out=ident, in_=id_d.bitcast(f32r))
            nc.sync.dma_start(out=identh, in_=idh_d)
            nc.sync.dma_start(out=b0q, in_=b0_d)
            nc.sync.dma_start(out=bmq, in_=bm_d)
            nc.sync.dma_start(out=blq, in_=bl_d)
            nc.sync.dma_start(out=bpq, in_=bp_d)

            for s in range(n_st):
                base = s * NT
                # --- entry: DMA natural fp16 tiles, PE-transpose to
                # feature-major fp32r
                xA = apool.tile([128, NT], f32r)
                xB = apool.tile([72, NT], f32r)
                for g in range(4):
                    nat = npool.tile([128, 200], f16, tag="nat")
                    nc.sync.dma_start(
                        out=nat, in_=x_d[base+g*128:base+(g+1)*128, :])
                    psA = pspool.tile([128, 128], f16, tag="ps")
                    nc.tensor.transpose(psA, nat[:, 0:128], identh)
                    nc.vector.tensor_copy(xA[:, g*128:(g+1)*128], psA)
                    psB = pspool.tile([72, 128], f16, tag="ps")
                    nc.tensor.transpose(psB, nat[:, 128:200], identh)
                    nc.vector.tensor_copy(xB[:, g*128:(g+1)*128], psB)

                # --- conv0 (dense 200->625)
                h = apool.tile([125, 5, NT], f32r, tag="h")
                for y in range(5):
                    ps = pspool.tile([125, NT], f32, tag="ps")
                    nc.tensor.matmul(ps, w0a[:, y*125:(y+1)*125], xA,
                                     start=True, stop=False)
                    nc.tensor.matmul(ps, w0b[:, y*125:(y+1)*125], xB,
                                     start=False, stop=True)
                    if y >= 3:  # balance eviction load ACT vs DVE
                        nc.vector.tensor_scalar(h[:, y, :], ps, b0q, 0.0,
                                                op0=ALU.add, op1=ALU.max)
                    else:
                        nc.scalar.activation(h[:, y, :], ps, AF.Relu, bias=b0q)

                # --- 6 mid layers (row-banded 625->625)
                for l in range(NUM_MID):
                    hn = apool.tile([125, 5, NT], f32r, tag="h")
                    for y in range(5):
                        bnd = _band(y)
                        bi = sum(len(_band(yy)) for yy in range(y))
                        ps = pspool.tile([125, NT], f32, tag="ps")
                        for j, y_in in enumerate(bnd):
                            nc.tensor.matmul(ps, wm[:, l, bi+j, :], h[:, y_in, :],
                                             start=(j == 0), stop=(j == len(bnd)-1))
                        if y >= 3:
                            nc.vector.tensor_scalar(hn[:, y, :], ps,
                                                    bmq[:, l:l+1], 0.0,
                                                    op0=ALU.add, op1=ALU.max)
                        else:
                            nc.scalar.activation(hn[:, y, :], ps, AF.Relu,
                                                 bias=bmq[:, l:l+1])
                    h = hn

                # --- last layer (625->150, logits, w-major cols)
                hl = apool.tile([75, 2, NT], f32r)
                for m in range(2):
                    ps = pspool.tile([75, NT], f32, tag="ps")
                    for k in range(5):
                        nc.tensor.matmul(ps, wl[:, k, m*75:(m+1)*75], h[:, k, :],
                                         start=(k == 0), stop=(k == 4))
                    nc.scalar.activation(hl[:, m, :], ps, AF.Identity,
                                         bias=blq[:, m:m+1])

                # --- post conv (colors: 75->18)
                colors = apool.tile([18, NT], f32r)
                psc = pspool.tile([18, NT], f32, tag="ps")
                nc.tensor.matmul(psc, wp, xA[0:75, :], start=True, stop=True)
                nc.scalar.activation(colors, psc, AF.Identity, bias=bpq)

                # --- tail: per 128-group, sample-major softmax + color mix
                for g in range(4):
                    gs = slice(g*128, (g+1)*128)
                    # fp32r matmul ISA restriction: innermost free n_step must
                    # be even on moving operand and dst -> pad 75 to 76.
                    tE0 = pspool.tile([128, 76], f32r, tag="ps")
                    nc.tensor.transpose(tE0, hl[:, 0, gs], ident[0:75, 0:76])
                    tE1 = pspool.tile([128, 76], f32r, tag="ps")
                    nc.tensor.transpose(tE1, hl[:, 1, gs], ident[0:75, 0:76])
                    E = tpool.tile([128, 150], f32, tag="E")
                    nc.scalar.activation(E[:, 0:75], tE0[:, 0:75], AF.Exp)
                    nc.scalar.activation(E[:, 75:150], tE1[:, 0:75], AF.Exp)
                    tC = pspool.tile([128, 18], f32r, tag="ps")
                    nc.tensor.transpose(tC, colors[:, gs], ident[0:18, 0:18])
                    colT = tpool.tile([128, 18], f32, tag="colT")
                    nc.scalar.activation(colT, tC, AF.Copy)

                    S = tpool.tile([128, 25], f32, tag="S")
                    nc.vector.tensor_reduce(
                        out=S, in_=E.rearrange("p (w q) -> p q w", w=6),
                        axis=mybir.AxisListType.X, op=ALU.add)
                    R = tpool.tile([128, 25], f32, tag="R")
                    nc.vector.reciprocal(R, S)

                    U = tpool.tile([128, 3, 25], f32, tag="U")
                    for c in range(3):
                        nc.vector.tensor_scalar_mul(
                            U[:, c, :], E[:, 0:25], colT[:, c*6:c*6+1])
                        for w in range(1, 6):
                            nc.vector.scalar_tensor_tensor(
                                out=U[:, c, :], in0=E[:, w*25:(w+1)*25],
                                scalar=colT[:, c*6+w:c*6+w+1], in1=U[:, c, :],
                                op0=ALU.mult, op1=ALU.add)
                    F = tpool.tile([128, 3, 25], f16, tag="F")
                    nc.vector.tensor_tensor(
                        out=F, in0=U,
                        in1=R.unsqueeze(1).broadcast_to([128, 3, 25]),
                        op=ALU.mult)
                    nc.sync.dma_start(
                        out=y_d[base+g*128:base+(g+1)*128, :],
                        in_=F.rearrange("p a b -> p (a b)"))

    nc.compile()
    return nc


def _prep_weights(w0, b0, wmid, bmid, wlast, blast, wpost, bpost):
    W0 = _densify_conv0(np.asarray(w0, np.float32))
    wm = np.zeros((125, NUM_MID, 13, 125), np.float32)
    for l in range(NUM_MID):
        blocks = _densify_mid(np.asarray(wmid[l], np.float32))
        for bi in range(13):
            wm[:, l, bi, :] = blocks[bi]
    Wl = _densify_last(np.asarray(wlast, np.float32))
    wl = np.ascontiguousarray(
        np.transpose(Wl.reshape(5, 125, 150), (1, 0, 2)))
    wp = np.ascontiguousarray(
        np.asarray(wpost, np.float32).reshape(18, 75).T)
    b0q = np.tile(np.asarray(b0, np.float32), 5)[:, None]
    bmq = np.stack([np.tile(np.asarray(bmid[l], np.float32), 5)
                    for l in range(NUM_MID)], axis=1)
    blq = np.asarray(blast, np.float32).repeat(25).reshape(2, 75).T
    bpq = np.asarray(bpost, np.float32)[:, None]
    return {
        "w0a": np.ascontiguousarray(W0[0:128]),
        "w0b": np.ascontiguousarray(W0[128:200]),
        "wm": wm, "wl": wl, "wp": wp,
        "ident": np.eye(128, dtype=np.float32),
        "identh": np.eye(128, dtype=np.float16),
        "b0q": np.ascontiguousarray(b0q), "bmq": np.ascontiguousarray(bmq),
        "blq": np.ascontiguousarray(blq), "bpq": bpq,
    }


def _get_ctx():
    """Build (once per process) the Bass module and a STABLE jitted runner."""
    if "ctx" in _CACHE:
        return _CACHE["ctx"]

    import jax
    import jax.numpy as jnp
    from jax.sharding import Mesh, PartitionSpec, NamedSharding
    from jax.experimental.shard_map import shard_map
    from concourse import bass2jax
    import concourse.mybir as mybir

    nc = _build(NPC)
    bass2jax.install_neuronx_cc_hook()
    assert nc.dbg_addr is None, "built with debug=False"

    partition_name = (nc.partition_id_tensor.name
                      if nc.partition_id_tensor is not None else None)
    in_names, out_names, out_avals = [], [], []
    for alloc in nc.m.functions[0].allocations:
        if not isinstance(alloc, mybir.MemoryLocationSet):
            continue
        name = alloc.memorylocations[0].name
        if alloc.kind == "ExternalInput":
            if name != partition_name:
                in_names.append(name)
        elif alloc.kind == "ExternalOutput":
            out_names.append(name)
            shape = tuple(alloc.tensor_shape)
            dtype = mybir.dt.np(alloc.dtype)
            out_avals.append(jax.core.ShapedArray(shape, dtype))
    n_params = len(in_names)
    all_in_names = list(in_names) + list(out_names)
    if partition_name is not None:
        all_in_names.append(partition_name)

    def _body(*args):
        operands = list(args)
        if partition_name is not None:
            operands.append(bass2jax.partition_id_tensor())
        outs = bass2jax._bass_exec_p.bind(
            *operands,
            out_avals=tuple(out_avals),
            in_names=tuple(all_in_names),
            out_names=tuple(out_names),
            lowering_input_output_aliases=(),
            sim_require_finite=True,
            sim_require_nnan=True,
            nc=nc,
        )
        return tuple(outs)

    devices = jax.devices()[:N_CORES]
    assert len(devices) == N_CORES
    mesh = Mesh(np.asarray(devices), ("core",))
    n_outs = len(out_names)
    in_specs = (PartitionSpec("core"),) * (n_params + n_outs)
    out_specs = (PartitionSpec("core"),) * n_outs
    # No donation: the kernel fully writes y, so the (NEFF-unbound) zero
    # buffers are allocated on-device once and reused every call.
    runner = jax.jit(
        shard_map(_body, mesh=mesh, in_specs=in_specs,
                  out_specs=out_specs, check_rep=False),
        keep_unused=True)
    shard = NamedSharding(mesh, PartitionSpec("core"))
    zeros = [
        jax.jit(
            (lambda aval: lambda: jnp.zeros(
                (N_CORES * aval.shape[0],) + tuple(aval.shape[1:]), aval.dtype
            ))(a),
            out_shardings=shard)()
        for a in out_avals
    ]
    for z in zeros:
        z.block_until_ready()
    ctx = {
        "nc": nc, "runner": runner, "zeros": zeros, "shard": shard,
        "in_names": in_names, "out_names": out_names, "out_avals": out_avals,
        "jax": jax,
    }
    _CACHE["ctx"] = ctx
    return ctx


def _stage_weights(ctx, wmap):
    """Replicate each weight across the 8 cores as a device-resident global
    array (sharded concat on axis 0), uploaded once and reused every call."""
    jax = ctx["jax"]
    staged = {}
    for k, v in wmap.items():
        g = np.ascontiguousarray(
            np.broadcast_to(v[None], (N_CORES,) + v.shape).reshape(
                (N_CORES * v.shape[0],) + v.shape[1:]))
        staged[k] = jax.device_put(g, ctx["shard"])
    for a in staged.values():
        a.block_until_ready()
    return staged


def _cast_f16(x):
    """f32 [B,8,5,5] -> contiguous f16 [B,200], threaded (numpy astype is
    single-core; 4 threads release the GIL in the C cast loop)."""
    x = x.reshape(B_TOTAL, 200)
    if x.dtype == np.float16:
        return np.ascontiguousarray(x)
    from concurrent.futures import ThreadPoolExecutor
    out = np.empty((B_TOTAL, 200), np.float16)
    nt = 4
    step = B_TOTAL // nt
    def cast(i):
        np.copyto(out[i*step:(i+1)*step], x[i*step:(i+1)*step],
                  casting="same_kind")
    if "castpool" not in _CACHE:
        _CACHE["castpool"] = ThreadPoolExecutor(nt)
    list(_CACHE["castpool"].map(cast, range(nt)))
    return out


def _weight_key(*arrs):
    h = hashlib.blake2b(digest_size=16)
    for a in arrs:
        h.update(np.ascontiguousarray(np.asarray(a, np.float32)).tobytes())
    return h.digest()


def kernel(input, w0, b0, wmid, bmid, wlast, blast, wpost, bpost, _trace=False):
    t0 = time.time()
    if _trace:
        return _kernel_traced(input, w0, b0, wmid, bmid, wlast, blast,
                              wpost, bpost)
    ctx = _get_ctx()
    _tlog("ctx ready", t0)

    key = _weight_key(w0, b0, wmid, bmid, wlast, blast, wpost, bpost)
    if _CACHE.get("wkey") != key:
        wmap = _prep_weights(w0, b0, wmid, bmid, wlast, blast, wpost, bpost)
        _tlog("weights densified", t0)
        _CACHE["weights"] = _stage_weights(ctx, wmap)
        _CACHE["wkey"] = key
        _tlog("weights staged to devices", t0)
    staged = _CACHE["weights"]

    x = _cast_f16(np.asarray(input))
    _tlog("input marshaled (fp16)", t0)

    jax = ctx["jax"]
    runner, zeros, shard = ctx["runner"], ctx["zeros"], ctx["shard"]
    outs = []
    for c in range(CHUNKS):
        xc = x[c*B_CHUNK:(c+1)*B_CHUNK] if CHUNKS > 1 else x
        xd = jax.device_put(xc, shard)
        args = [xd if name == "x" else staged[name] for name in ctx["in_names"]]
        outs.append(runner(*args, *zeros))
    _tlog("all chunks dispatched", t0)
    parts = [np.asarray(o[0]) for o in outs]
    _tlog("output fetched", t0)
    out = parts[0] if CHUNKS == 1 else np.concatenate(parts, axis=0)
    return out.astype(np.float32).reshape(B_TOTAL, 3, 5, 5)


def _kernel_traced(input, w0, b0, wmid, bmid, wlast, blast, wpost, bpost):
    """Legacy library path (per-call compile) — only used for --trace runs."""
    from concourse import bass_utils
    if "nc_trace" not in _CACHE:
        _CACHE["nc_trace"] = _build(N_PER_CORE)
    nc = _CACHE["nc_trace"]
    wmap = _prep_weights(w0, b0, wmid, bmid, wlast, blast, wpost, bpost)
    x = np.asarray(input).astype(np.float16).reshape(B_TOTAL, 200)
    in_maps = []
    for c in range(N_CORES):
        m = dict(wmap)
        m["x"] = np.ascontiguousarray(x[c*N_PER_CORE:(c+1)*N_PER_CORE])
        in_maps.append(m)
    res = bass_utils.run_bass_kernel_spmd(
        nc, in_maps, core_ids=list(range(N_CORES)), trace=True)
    out = np.concatenate([res.results[c]["y"] for c in range(N_CORES)], axis=0)
    _CACHE["last_result"] = res
    return out.astype(np.float32).reshape(B_TOTAL, 3, 5, 5)


# revision 8
# speedup vs baseline: 4.2292x; 1.0772x over previous
"""KPCNN (kernel-predicting CNN) Trainium2 Bass kernel.

Device strategy (B=32768, 8 cores, pure data parallel):
 - All convs on 5x5 spatial are reformulated as dense matmuls over flattened
   (pixel, channel) feature vectors, row-banded by output image row so each
   125-wide output chunk contracts only the 2-3 input row chunks in its 3x3
   band (13 [125x125] blocks per 25->25 layer instead of 25).
 - Activations are feature-major [feat, batch] in SBUF, fp32r (TF32-like,
   full PE rate at N>=256), PSUM accumulate fp32.
 - Entry: PE-transpose of naturally-DMA'd [128 samples, 200 feat] fp16 tiles.
 - Tail (softmax over 6 predicted weights + per-pixel color mix) runs
   sample-major after PE-transposing back, on DVE/ACT; result stored fp16.

Host dispatch (the actual bottleneck — the axon tunnel moves ~49MB/s H2D,
~30MB/s D2H, with ~70ms RTT per synchronous dispatch):
 - One STABLE jax.jit(shard_map(bass_exec)) built per process and cached
   (the library's run_bass_kernel_spmd rebuilds its closure per call, which
   forces a retrace + XLA + BIR->NEFF recompile on every invocation).
 - Weights are densified once, replicated, and kept device-resident.
 - I/O travels as fp16 (input 13MB down, output 4.9MB up instead of 26/9.8
   fp32); fp16's 10-bit mantissa matches the fp32r compute precision.
 - The NEFF's output buffer is fully written by the kernel, so the zero
   output-donation buffers the library path re-ships per call are created
   on-device once and reused (no donation).
 - Optional batch chunking (KERNEL_CHUNKS) pipelines H2D/exec/D2H.
"""
import sys
sys.path.insert(0, '/opt/trn_rl_repo')
import os
import time
import hashlib
import numpy as np

B_TOTAL = 32768
N_CORES = 8
N_PER_CORE = B_TOTAL // N_CORES   # 4096
NT = 512                          # samples per supertile
NUM_MID = 6
CHUNKS = int(os.environ.get("KERNEL_CHUNKS", "1"))
NPC = N_PER_CORE // CHUNKS        # samples per core per chunk
B_CHUNK = B_TOTAL // CHUNKS

_CACHE = {}
_TIMING = os.environ.get("KERNEL_TIMING", "") != ""


def _tlog(msg, t0):
    if _TIMING:
        print(f"[kernel.py {time.time()-t0:8.3f}s] {msg}", file=sys.stderr, flush=True)


def _band(y):
    return [yi for yi in (y - 1, y, y + 1) if 0 <= yi <= 4]


def _densify_mid(w):
    """w [25,25,3,3] OIHW -> [13,125,125] blocks (lhsT: [q_in, q_out])."""
    blocks = np.zeros((13, 125, 125), np.float32)
    bi = 0
    for y_out in range(5):
        for y_in in _band(y_out):
            dy = y_in - y_out
            for x_in in range(5):
                for x_out in range(5):
                    dx = x_in - x_out
                    if abs(dx) <= 1:
                        blocks[bi, x_in*25:(x_in+1)*25, x_out*25:(x_out+1)*25] = \
                            w[:, :, dy+1, dx+1].T
            bi += 1
    return blocks


def _densify_conv0(w):
    """w [25,8,3,3] -> [200,625]: row c_in*25+y_in*5+x_in, col y_out*125+x_out*25+c_out."""
    W = np.zeros((200, 625), np.float32)
    for y_in in range(5):
        for x_in in range(5):
            for y_out in range(5):
                dy = y_in - y_out
                if abs(dy) > 1:
                    continue
                for x_out in range(5):
                    dx = x_in - x_out
                    if abs(dx) > 1:
                        continue
                    for c_in in range(8):
                        W[c_in*25 + y_in*5 + x_in,
                          y_out*125 + x_out*25:y_out*125 + x_out*25 + 25] = \
                            w[:, c_in, dy+1, dx+1]
    return W


def _densify_last(w):
    """w [6,25,3,3] -> [625,150]: row y_in*125+x_in*25+c_in, col wi*25+y_out*5+x_out."""
    W = np.zeros((625, 150), np.float32)
    for y_in in range(5):
        for x_in in range(5):
            for y_out in range(5):
                dy = y_in - y_out
                if abs(dy) > 1:
                    continue
                for x_out in range(5):
                    dx = x_in - x_out
                    if abs(dx) > 1:
                        continue
                    for c_in in range(25):
                        for wi in range(6):
                            W[y_in*125 + x_in*25 + c_in, wi*25 + y_out*5 + x_out] = \
                                w[wi, c_in, dy+1, dx+1]
    return W


def _build(npc):
    import concourse.bass as bass
    from concourse import bacc
    import concourse.tile as tile
    import concourse.mybir as mybir

    dt = mybir.dt
    AF = mybir.ActivationFunctionType
    ALU = mybir.AluOpType

    nc = bacc.Bacc("TRN2", target_bir_lowering=False, debug=False)

    f32, f32r, f16 = dt.float32, dt.float32r, dt.float16
    n_st = npc // NT
    assert npc % NT == 0
    x_d = nc.dram_tensor("x", [npc, 200], f16, kind="ExternalInput").ap()
    y_d = nc.dram_tensor("y", [npc, 75], f16, kind="ExternalOutput").ap()
    w0a_d = nc.dram_tensor("w0a", [128, 625], f32, kind="ExternalInput").ap()
    w0b_d = nc.dram_tensor("w0b", [72, 625], f32, kind="ExternalInput").ap()
    wm_d = nc.dram_tensor("wm", [125, NUM_MID, 13, 125], f32, kind="ExternalInput").ap()
    wl_d = nc.dram_tensor("wl", [125, 5, 150], f32, kind="ExternalInput").ap()
    wp_d = nc.dram_tensor("wp", [75, 18], f32, kind="ExternalInput").ap()
    id_d = nc.dram_tensor("ident", [128, 128], f32, kind="ExternalInput").ap()
    idh_d = nc.dram_tensor("identh", [128, 128], f16, kind="ExternalInput").ap()
    b0_d = nc.dram_tensor("b0q", [125, 1], f32, kind="ExternalInput").ap()
    bm_d = nc.dram_tensor("bmq", [125, NUM_MID], f32, kind="ExternalInput").ap()
    bl_d = nc.dram_tensor("blq", [75, 2], f32, kind="ExternalInput").ap()
    bp_d = nc.dram_tensor("bpq", [18, 1], f32, kind="ExternalInput").ap()

    with tile.TileContext(nc) as tc:
        with tc.tile_pool(name="wpool", bufs=1) as wpool, \
             tc.tile_pool(name="apool", bufs=3) as apool, \
             tc.tile_pool(name="npool", bufs=6) as npool, \
             tc.tile_pool(name="tpool", bufs=6) as tpool, \
             tc.tile_pool(name="pspool", bufs=8, space="PSUM") as pspool:

        # --- load weights (resident in SBUF for the whole kernel)
            w0a = wpool.tile([128, 625], f32r)
            w0b = wpool.tile([72, 625], f32r)
            wm = wpool.tile([125, NUM_MID, 13, 125], f32r)
            wl = wpool.tile([125, 5, 150], f32r)
            wp = wpool.tile([75, 18], f32r)
            ident = wpool.tile([128, 128], f32r)
            identh = wpool.tile([128, 128], f16)
            b0q = wpool.tile([125, 1], f32)
            bmq = wpool.tile([125, NUM_MID], f32)
            blq = wpool.tile([75, 2], f32)
            bpq = wpool.tile([18, 1], f32)
            nc.sync.dma_start(out=w0a, in_=w0a_d.bitcast(f32r))
            nc.sync.dma_start(out=w0b, in_=w0b_d.bitcast(f32r))
            nc.sync.dma_start(out=wm, in_=wm_d.bitcast(f32r))
            nc.sync.dma_start(out=wl, in_=wl_d.bitcast(f32r))
            nc.sync.dma_start(out=wp, in_=wp_d.bitcast(f32r))
            nc.sync.dma_start(out=ident, in_=id_d.bitcast(f32r))
            nc.sync.dma_start(out=identh, in_=idh_d)
            nc.sync.dma_start(out=b0q, in_=b0_d)
            nc.sync.dma_start(out=bmq, in_=bm_d)
            nc.sync.dma_start(out=blq, in_=bl_d)
            nc.sync.dma_start(out=bpq, in_=bp_d)

            for s in range(n_st):
                base = s * NT
                # --- entry: DMA natural fp16 tiles, PE-transpose to
                # feature-major fp32r
                xA = apool.tile([128, NT], f32r)
                xB = apool.tile([72, NT], f32r)
                for g in range(4):
                    nat = npool.tile([128, 200], f16, tag="nat")
                    nc.sync.dma_start(
                        out=nat, in_=x_d[base+g*128:base+(g+1)*128, :])
                    psA = pspool.tile([128, 128], f16, tag="ps")
                    nc.tensor.transpose(psA, nat[:, 0:128], identh)
                    nc.vector.tensor_copy(xA[:, g*128:(g+1)*128], psA)
                    psB = pspool.tile([72, 128], f16, tag="ps")
                    nc.tensor.transpose(psB, nat[:, 128:200], identh)
                    nc.vector.tensor_copy(xB[:, g*128:(g+1)*128], psB)

                # --- conv0 (dense 200->625)
                h = apool.tile([125, 5, NT], f32r, tag="h")
                for y in range(5):
                    ps = pspool.tile([125, NT], f32, tag="ps")
                    nc.tensor.matmul(ps, w0a[:, y*125:(y+1)*125], xA,
                                     start=True, stop=False)
                    nc.tensor.matmul(ps, w0b[:, y*125:(y+1)*125], xB,
                                     start=False, stop=True)
                    if y >= 3:  # balance eviction load ACT vs DVE
                        nc.vector.tensor_scalar(h[:, y, :], ps, b0q, 0.0,
                                                op0=ALU.add, op1=ALU.max)
                    else:
                        nc.scalar.activation(h[:, y, :], ps, AF.Relu, bias=b0q)

                # --- 6 mid layers (row-banded 625->625)
                for l in range(NUM_MID):
                    hn = apool.tile([125, 5, NT], f32r, tag="h")
                    for y in range(5):
                        bnd = _band(y)
                        bi = sum(len(_band(yy)) for yy in range(y))
                        ps = pspool.tile([125, NT], f32, tag="ps")
                        for j, y_in in enumerate(bnd):
                            nc.tensor.matmul(ps, wm[:, l, bi+j, :], h[:, y_in, :],
                                             start=(j == 0), stop=(j == len(bnd)-1))
                        if y >= 3:
                            nc.vector.tensor_scalar(hn[:, y, :], ps,
                                                    bmq[:, l:l+1], 0.0,
                                                    op0=ALU.add, op1=ALU.max)
                        else:
                            nc.scalar.activation(hn[:, y, :], ps, AF.Relu,
                                                 bias=bmq[:, l:l+1])
                    h = hn

                # --- last layer (625->150, logits, w-major cols)
                hl = apool.tile([75, 2, NT], f32r)
                for m in range(2):
                    ps = pspool.tile([75, NT], f32, tag="ps")
                    for k in range(5):
                        nc.tensor.matmul(ps, wl[:, k, m*75:(m+1)*75], h[:, k, :],
                                         start=(k == 0), stop=(k == 4))
                    nc.scalar.activation(hl[:, m, :], ps, AF.Identity,
                                         bias=blq[:, m:m+1])

                # --- post conv (colors: 75->18)
                colors = apool.tile([18, NT], f32r)
                psc = pspool.tile([18, NT], f32, tag="ps")
                nc.tensor.matmul(psc, wp, xA[0:75, :], start=True, stop=True)
                nc.scalar.activation(colors, psc, AF.Identity, bias=bpq)

                # --- tail: per 128-group, sample-major softmax + color mix
                for g in range(4):
                    gs = slice(g*128, (g+1)*128)
                    # fp32r matmul ISA restriction: innermost free n_step must
                    # be even on moving operand and dst -> pad 75 to 76.
                    tE0 = pspool.tile([128, 76], f32r, tag="ps")
                    nc.tensor.transpose(tE0, hl[:, 0, gs], ident[0:75, 0:76])
                    tE1 = pspool.tile([128, 76], f32r, tag="ps")
                    nc.tensor.transpose(tE1, hl[:, 1, gs], ident[0:75, 0:76])
                    E = tpool.tile([128, 150], f32, tag="E")
                    nc.scalar.activation(E[:, 0:75], tE0[:, 0:75], AF.Exp)
                    nc.scalar.activation(E[:, 75:150], tE1[:, 0:75], AF.Exp)
                    tC = pspool.tile([128, 18], f32r, tag="ps")
                    nc.tensor.transpose(tC, colors[:, gs], ident[0:18, 0:18])
                    colT = tpool.tile([128, 18], f32, tag="colT")
                    nc.scalar.activation(colT, tC, AF.Copy)

                    S = tpool.tile([128, 25], f32, tag="S")
                    nc.vector.tensor_reduce(
                        out=S, in_=E.rearrange("p (w q) -> p q w", w=6),
                        axis=mybir.AxisListType.X, op=ALU.add)
                    R = tpool.tile([128, 25], f32, tag="R")
                    nc.vector.reciprocal(R, S)

                    U = tpool.tile([128, 3, 25], f32, tag="U")
                    for c in range(3):
                        nc.vector.tensor_scalar_mul(
                            U[:, c, :], E[:, 0:25], colT[:, c*6:c*6+1])
                        for w in range(1, 6):
                            nc.vector.scalar_tensor_tensor(
                                out=U[:, c, :], in0=E[:, w*25:(w+1)*25],
                                scalar=colT[:, c*6+w:c*6+w+1], in1=U[:, c, :],
                                op0=ALU.mult, op1=ALU.add)
                    F = tpool.tile([128, 3, 25], f16, tag="F")
                    nc.vector.tensor_tensor(
                        out=F, in0=U,
                        in1=R.unsqueeze(1).broadcast_to([128, 3, 25]),
                        op=ALU.mult)
                    nc.sync.dma_start(
                        out=y_d[base+g*128:base+(g+1)*128, :],
                        in_=F.rearrange("p a b -> p (a b)"))

    nc.compile()
    return nc


def _prep_weights(w0, b0, wmid, bmid, wlast, blast, wpost, bpost):
    W0 = _densify_conv0(np.asarray(w0, np.float32))
    wm = np.zeros((125, NUM_MID, 13, 125), np.float32)
    for l in range(NUM_MID):
        blocks = _densify_mid(np.asarray(wmid[l], np.float32))
        for bi in range(13):
            wm[:, l, bi, :] = blocks[bi]
    Wl = _densify_last(np.asarray(wlast, np.float32))
    wl = np.ascontiguousarray(
        np.transpose(Wl.reshape(5, 125, 150), (1, 0, 2)))
    wp = np.ascontiguousarray(
        np.asarray(wpost, np.float32).reshape(18, 75).T)
    b0q = np.tile(np.asarray(b0, np.float32), 5)[:, None]
    bmq = np.stack([np.tile(np.asarray(bmid[l], np.float32), 5)
                    for l in range(NUM_MID)], axis=1)
    blq = np.asarray(blast, np.float32).repeat(25).reshape(2, 75).T
    bpq = np.asarray(bpost, np.float32)[:, None]
    return {
        "w0a": np.ascontiguousarray(W0[0:128]),
        "w0b": np.ascontiguousarray(W0[128:200]),
        "wm": wm, "wl": wl, "wp": wp,
        "ident": np.eye(128, dtype=np.float32),
        "identh": np.eye(128, dtype=np.float16),
        "b0q": np.ascontiguousarray(b0q), "bmq": np.ascontiguousarray(bmq),
        "blq": np.ascontiguousarray(blq), "bpq": bpq,
    }


def _get_ctx():
    """Build (once per process) the Bass module and a STABLE jitted runner."""
    if "ctx" in _CACHE:
        return _CACHE["ctx"]

    import jax
    import jax.numpy as jnp
    from jax.sharding import Mesh, PartitionSpec, NamedSharding
    from jax.experimental.shard_map import shard_map
    from concourse import bass2jax
    import concourse.mybir as mybir

    nc = _build(NPC)
    bass2jax.install_neuronx_cc_hook()
    assert nc.dbg_addr is None, "built with debug=False"

    partition_name = (nc.partition_id_tensor.name
                      if nc.partition_id_tensor is not None else None)
    in_names, out_names, out_avals = [], [], []
    for alloc in nc.m.functions[0].allocations:
        if not isinstance(alloc, mybir.MemoryLocationSet):
            continue
        name = alloc.memorylocations[0].name
        if alloc.kind == "ExternalInput":
            if name != partition_name:
                in_names.append(name)
        elif alloc.kind == "ExternalOutput":
            out_names.append(name)
            shape = tuple(alloc.tensor_shape)
            dtype = mybir.dt.np(alloc.dtype)
            out_avals.append(jax.core.ShapedArray(shape, dtype))
    n_params = len(in_names)
    all_in_names = list(in_names) + list(out_names)
    if partition_name is not None:
        all_in_names.append(partition_name)

    def _body(*args):
        operands = list(args)
        if partition_name is not None:
            operands.append(bass2jax.partition_id_tensor())
        outs = bass2jax._bass_exec_p.bind(
            *operands,
            out_avals=tuple(out_avals),
            in_names=tuple(all_in_names),
            out_names=tuple(out_names),
            lowering_input_output_aliases=(),
            sim_require_finite=True,
            sim_require_nnan=True,
            nc=nc,
        )
        return tuple(outs)

    devices = jax.devices()[:N_CORES]
    assert len(devices) == N_CORES
    mesh = Mesh(np.asarray(devices), ("core",))
    n_outs = len(out_names)
    in_specs = (PartitionSpec("core"),) * (n_params + n_outs)
    out_specs = (PartitionSpec("core"),) * n_outs
    # No donation: the kernel fully writes y, so the (NEFF-unbound) zero
    # buffers are allocated on-device once and reused every call.
    runner = jax.jit(
        shard_map(_body, mesh=mesh, in_specs=in_specs,
                  out_specs=out_specs, check_rep=False),
        keep_unused=True)
    shard = NamedSharding(mesh, PartitionSpec("core"))
    zeros = [
        jax.jit(
            (lambda aval: lambda: jnp.zeros(
                (N_CORES * aval.shape[0],) + tuple(aval.shape[1:]), aval.dtype
            ))(a),
            out_shardings=shard)()
        for a in out_avals
    ]
    for z in zeros:
        z.block_until_ready()
    ctx = {
        "nc": nc, "runner": runner, "zeros": zeros, "shard": shard,
        "in_names": in_names, "out_names": out_names, "out_avals": out_avals,
        "jax": jax,
    }
    _CACHE["ctx"] = ctx
    return ctx


def _stage_weights(ctx, wmap):
    """Replicate each weight across the 8 cores as a device-resident global
    array (sharded concat on axis 0), uploaded once and reused every call."""
    jax = ctx["jax"]
    staged = {}
    for k, v in wmap.items():
        g = np.ascontiguousarray(
            np.broadcast_to(v[None], (N_CORES,) + v.shape).reshape(
                (N_CORES * v.shape[0],) + v.shape[1:]))
        staged[k] = jax.device_put(g, ctx["shard"])
    for a in staged.values():
        a.block_until_ready()
    return staged


def _cast_f16(x):
    """f32 [B,8,5,5] -> contiguous f16 [B,200] via jax-cpu (F16C SIMD;
    numpy's half cast is scalar and ~4x slower)."""
    x = x.reshape(B_TOTAL, 200)
    if x.dtype == np.float16:
        return np.ascontiguousarray(x)
    import jax
    import jax.numpy as jnp
    if "cast_dn" not in _CACHE:
        _CACHE["cast_dn"] = jax.jit(
            lambda a: a.astype(jnp.float16), backend="cpu")
    return np.asarray(_CACHE["cast_dn"](x))


def _cast_f32(y):
    """f16 [B,75] -> f32, via jax-cpu SIMD."""
    import jax
    import jax.numpy as jnp
    if "cast_up" not in _CACHE:
        _CACHE["cast_up"] = jax.jit(
            lambda a: a.astype(jnp.float32), backend="cpu")
    return np.asarray(_CACHE["cast_up"](y))


def _weight_key(*arrs):
    h = hashlib.blake2b(digest_size=16)
    for a in arrs:
        h.update(np.ascontiguousarray(np.asarray(a, np.float32)).tobytes())
    return h.digest()


def kernel(input, w0, b0, wmid, bmid, wlast, blast, wpost, bpost, _trace=False):
    t0 = time.time()
    if _trace:
        return _kernel_traced(input, w0, b0, wmid, bmid, wlast, blast,
                              wpost, bpost)
    ctx = _get_ctx()
    _tlog("ctx ready", t0)

    key = _weight_key(w0, b0, wmid, bmid, wlast, blast, wpost, bpost)
    if _CACHE.get("wkey") != key:
        wmap = _prep_weights(w0, b0, wmid, bmid, wlast, blast, wpost, bpost)
        _tlog("weights densified", t0)
        _CACHE["weights"] = _stage_weights(ctx, wmap)
        _CACHE["wkey"] = key
        _tlog("weights staged to devices", t0)
    staged = _CACHE["weights"]

    x = _cast_f16(np.asarray(input))
    _tlog("input marshaled (fp16)", t0)

    jax = ctx["jax"]
    runner, zeros, shard = ctx["runner"], ctx["zeros"], ctx["shard"]
    outs = []
    for c in range(CHUNKS):
        xc = x[c*B_CHUNK:(c+1)*B_CHUNK] if CHUNKS > 1 else x
        xd = jax.device_put(xc, shard)
        args = [xd if name == "x" else staged[name] for name in ctx["in_names"]]
        outs.append(runner(*args, *zeros))
    _tlog("all chunks dispatched", t0)
    parts = [np.asarray(o[0]) for o in outs]
    _tlog("output fetched", t0)
    out = parts[0] if CHUNKS == 1 else np.concatenate(parts, axis=0)
    out = _cast_f32(out)
    _tlog("output upcast", t0)
    return out.reshape(B_TOTAL, 3, 5, 5)


def _kernel_traced(input, w0, b0, wmid, bmid, wlast, blast, wpost, bpost):
    """Legacy library path (per-call compile) — only used for --trace runs."""
    from concourse import bass_utils
    if "nc_trace" not in _CACHE:
        _CACHE["nc_trace"] = _build(N_PER_CORE)
    nc = _CACHE["nc_trace"]
    wmap = _prep_weights(w0, b0, wmid, bmid, wlast, blast, wpost, bpost)
    x = np.asarray(input).astype(np.float16).reshape(B_TOTAL, 200)
    in_maps = []
    for c in range(N_CORES):
        m = dict(wmap)
        m["x"] = np.ascontiguousarray(x[c*N_PER_CORE:(c+1)*N_PER_CORE])
        in_maps.append(m)
    res = bass_utils.run_bass_kernel_spmd(
        nc, in_maps, core_ids=list(range(N_CORES)), trace=True)
    out = np.concatenate([res.results[c]["y"] for c in range(N_CORES)], axis=0)
    _CACHE["last_result"] = res
    return out.astype(np.float32).reshape(B_TOTAL, 3, 5, 5)
